# revision 11
# baseline (speedup 1.0000x reference)
"""CRF forward-backward marginals on 8 Trainium2 NeuronCores.

Strategy (hardcoded for B=64, T=512, D=1024, K=32, 8 cores):
  - Data-parallel over batch: core i handles batches [8i, 8i+8).
  - Emissions: E'^T[k, (b,t)] = exp(x @ (W - W[:,0]) + (b - b[0])) via
    PE-transpose of x tiles + fp32r accumulate matmul + ACT Exp.
    (Subtracting the k=0 column bounds the per-(b,t) scale; marginals are
    invariant to per-(b,t) positive rescalings.)
  - Forward/backward recursions in scaled probability space with
    eUn = exp(U)/(K*e) (per-step-constant invariant):
      fwd:  p_t = (p_{t-1} @ eUn) * E'_t          p_0 = E'_0
      bwd:  w_t = (w_{t+1} @ eUn^T) * E'_t        w_{T-1} = E'_{T-1}
      marginal_t = rownorm(v_t * w_t),  v_t = p_{t-1} @ eUn  (v_0 = 1)
    Time-parallelized over 32 chunks of 16 steps with 8 burn-in steps
    (the transition kernel contracts in the Hilbert metric ~0.3x/step, so 8
    steps reach fp32 accuracy); fwd chunk 0 / bwd chunk 31 exactly re-init.
  - Combine, PE-transpose back to [t, k] layout, rownorm, DMA out.
"""

import os
import sys

import numpy as np

sys.path.insert(0, "/opt/trn_rl_repo")

import concourse.bass as bass  # noqa: E402
import concourse.bacc as bacc  # noqa: E402
import concourse.mybir as mybir  # noqa: E402
from concourse import tile  # noqa: E402
from concourse.masks import make_identity  # noqa: E402

B, T, D, K = 64, 512, 1024, 32
NCORES = 8
BL = B // NCORES            # 8 batches per core
ROWS = BL * T               # 4096 rows per core
S_CH = 16                   # chunk length
V_BI = 8                    # burn-in positions
C_CH = T // S_CH            # 32 chunks
CHAINS = BL * C_CH          # 256 parallel chains
POS = S_CH + V_BI           # 24 scan positions per direction
TP = 536                    # padded time width: [0,8) pad | t+8 | [520,536) pad
LOG_CU = -(np.log(K) + 1.0)  # log(1/(K*e)) folded into exp(U)

f32 = mybir.dt.float32
f32r = mybir.dt.float32r
AX = mybir.AxisListType
ALU = mybir.AluOpType
ACTF = mybir.ActivationFunctionType

USE_F32R = True   # reduced-precision PE matmul mode (4x faster)


def _r(ap):
    return ap.bitcast(f32r) if USE_F32R else ap


def build_nc():
    nc = bacc.Bacc("TRN2", target_bir_lowering=False)
    x_h = nc.declare_dram_parameter("x", [ROWS, D], f32, isOutput=False)
    w_h = nc.declare_dram_parameter("W", [D, K], f32, isOutput=False)
    u_h = nc.declare_dram_parameter("U", [K, K], f32, isOutput=False)
    b_h = nc.declare_dram_parameter("b", [1, K], f32, isOutput=False)
    o_h = nc.declare_dram_parameter("out", [ROWS, K], f32, isOutput=True)

    with tile.TileContext(nc) as tc:
        with (
            tc.tile_pool(name="const", bufs=1) as cpool,
            tc.tile_pool(name="stores", bufs=1) as spool,
        ):
            # ---------------- constants / small inputs ----------------
            id128 = cpool.tile([128, 128], f32)
            make_identity(nc, id128[:])

            w_raw = cpool.tile([128, 8, K], f32)
            nc.sync.dma_start(w_raw[:], w_h.ap().rearrange("(n p) k -> p n k", p=128))
            wn = cpool.tile([128, 8, K], f32r)
            for n in range(8):
                nc.vector.tensor_scalar_sub(wn[:, n, :], w_raw[:, n, :],
                                            w_raw[:, n, 0:1])

            u_nat = cpool.tile([K, K], f32)
            nc.sync.dma_start(u_nat[:], u_h.ap())
            u_t = cpool.tile([K, K], f32)
            nc.vector.transpose(u_t[:], u_nat[:])
            eUn = cpool.tile([K, K], f32r)
            nc.scalar.activation(eUn[:], u_nat[:], ACTF.Exp)
            eUnT = cpool.tile([K, K], f32r)
            nc.scalar.activation(eUnT[:], u_t[:], ACTF.Exp)

            b_nat = cpool.tile([1, K], f32)
            nc.sync.dma_start(b_nat[:], b_h.ap())
            one_sb = cpool.tile([1, 1], f32)
            nc.vector.memset(one_sb[:], 1.0)
            ones_row = cpool.tile([1, K], f32)
            nc.vector.memset(ones_row[:], 1.0)
            bn = cpool.tile([K, 1], f32)
            with tc.tile_pool(name="ps_b", bufs=2, space="PSUM") as ps_b_pool:
                bt_ps = ps_b_pool.tile([K, 1], f32, tag="bt")
                nc.tensor.matmul(bt_ps[:], b_nat[:], one_sb[:], start=True, stop=True)
                b0_ps = ps_b_pool.tile([K, 1], f32, tag="b0")
                nc.tensor.matmul(b0_ps[:], ones_row[:], b_nat[:, 0:1],
                                 start=True, stop=True)
                bt_sb = cpool.tile([K, 1], f32)
                nc.vector.tensor_copy(bt_sb[:], bt_ps[:])
                nc.vector.scalar_tensor_tensor(
                    bt_sb[:], bt_sb[:], float(LOG_CU), b0_ps[:],
                    op0=ALU.add, op1=ALU.subtract)
                nc.scalar.activation(bn[:], bt_sb[:], ACTF.Copy)

            # ---------------- big stores ----------------
            E = spool.tile([K, BL, TP], f32)     # E'_t at offset t+8
            P = spool.tile([K, BL, TP], f32r)     # fwd states p_t
            Vst = spool.tile([K, BL, TP], f32)   # fwd pre-multiply v_t
            Wst = spool.tile([K, BL, TP], f32r)   # bwd states w_t

            nc.gpsimd.memset(E[:, :, 0:8], float(np.exp(LOG_CU)))
            nc.gpsimd.memset(E[:, :, 520:TP], float(np.exp(LOG_CU)))

            # ---------------- emission phase ----------------
            with (
                tc.tile_pool(name="xin", bufs=2) as xpool,
                tc.tile_pool(name="xt", bufs=3) as xtpool,
                tc.tile_pool(name="ps_t", bufs=2, space="PSUM") as ps_t_pool,
                tc.tile_pool(name="ps_e", bufs=2, space="PSUM") as ps_e_pool,
            ):
                for st in range(BL):
                    x_sb = xpool.tile([128, 4, D], f32)
                    nc.sync.dma_start(
                        x_sb[:],
                        x_h.ap()[st * 512:(st + 1) * 512, :].rearrange(
                            "(rb p) d -> p rb d", p=128),
                    )
                    e_ps = ps_e_pool.tile([K, 512], f32)
                    for db in range(8):
                        ps_t = ps_t_pool.tile([128, 512], f32)
                        for rb in range(4):
                            nc.tensor.transpose(
                                ps_t[:, rb * 128:(rb + 1) * 128],
                                x_sb[:, rb, db * 128:(db + 1) * 128],
                                id128[:],
                            )
                        xt_sb = xtpool.tile([128, 512], f32r)
                        if db % 2 == 0:
                            nc.vector.tensor_copy(xt_sb[:], ps_t[:])
                        else:
                            nc.scalar.activation(xt_sb[:], ps_t[:], ACTF.Copy)
                        nc.tensor.matmul(
                            e_ps[:], wn[:, db, :], xt_sb[:],
                            start=(db == 0), stop=(db == 7),
                        )
                    nc.scalar.activation(E[:, st, 8:520], e_ps[:], ACTF.Exp,
                                         bias=bn[:, 0:1])

            # ---------------- scans ----------------
            def sl(buf, off):
                return buf[:, :, off:off + S_CH * (C_CH - 1) + 1:S_CH]

            with tc.tile_pool(name="ps_s", bufs=3, space="PSUM") as ps_s_pool:
                for s in range(POS):
                    if s == 0:
                        # burn-in init: state := E-slice (any positive init works)
                        nc.vector.tensor_copy(sl(P, 0), sl(E, 0))
                        nc.vector.tensor_copy(sl(Wst, 31), sl(E, 31))
                        continue
                    # forward: P[., 16c+s] = (P[., 16c+s-1] @ eUn) * E[., 16c+s]
                    psA = ps_s_pool.tile([K, CHAINS], f32, tag="psA")
                    nc.tensor.matmul(psA[:], eUn[:], sl(P, s - 1).opt(),
                                     start=True, stop=True)
                    psA3 = psA[:].rearrange("k (b c) -> k b c", b=BL)
                    nc.vector.tensor_tensor(sl(P, s), psA3, sl(E, s), op=ALU.mult)
                    if s >= V_BI:
                        nc.scalar.activation(sl(Vst, s), psA3, ACTF.Copy)
                    # backward: Wst[., 16c+31-s] = (Wst[., 16c+32-s] @ eUnT) * E[., 16c+31-s]
                    psB = ps_s_pool.tile([K, CHAINS], f32, tag="psB")
                    nc.tensor.matmul(psB[:], eUnT[:], sl(Wst, 32 - s).opt(),
                                     start=True, stop=True)
                    psB3 = psB[:].rearrange("k (b c) -> k b c", b=BL)
                    nc.vector.tensor_tensor(sl(Wst, 31 - s), psB3, sl(E, 31 - s),
                                            op=ALU.mult)
                    if s == V_BI:
                        # exact re-inits once burn-in is done
                        nc.vector.tensor_copy(P[:, :, 8], E[:, :, 8])      # p_0=E'_0
                        nc.vector.memset(Vst[:, :, 8], 1.0)                # v_0=1
                        nc.vector.tensor_copy(Wst[:, :, 519], E[:, :, 519])  # w_last

            # ---------------- combine: M = Vst * Wst (into Vst) ----------------
            for q in range(4):
                lo = 8 + q * 128
                nc.vector.tensor_tensor(Vst[:, :, lo:lo + 128],
                                        Vst[:, :, lo:lo + 128],
                                        Wst[:, :, lo:lo + 128].bitcast(f32), op=ALU.mult)

            # ---------------- transpose + rownorm + out ----------------
            with (
                tc.tile_pool(name="outsb", bufs=3) as opool,
                tc.tile_pool(name="ps_o", bufs=2, space="PSUM") as ps_o_pool,
            ):
                for st in range(BL):
                    ps_o = ps_o_pool.tile([128, 4, K], f32)
                    for q in range(4):
                        nc.tensor.transpose(
                            ps_o[:, q, :],
                            Vst[:, st, 8 + q * 128:8 + (q + 1) * 128],
                            id128[:K, :K])
                    rs = opool.tile([128, 4], f32, tag="rs")
                    nc.vector.tensor_reduce(rs[:], ps_o[:], axis=AX.X, op=ALU.add)
                    rc = opool.tile([128, 4], f32, tag="rc")
                    nc.vector.reciprocal(rc[:], rs[:])
                    o_sb = opool.tile([128, 4, K], f32, tag="osb")
                    nc.vector.tensor_tensor(o_sb[:], ps_o[:],
                                            rc[:].to_broadcast((128, 4, K)),
                                            op=ALU.mult)
                    nc.sync.dma_start(
                        o_h.ap()[st * 512:(st + 1) * 512, :].rearrange(
                            "(q p) k -> p q k", p=128),
                        o_sb[:])
    nc.finalize()
    return nc


_NC_CACHE = {}


def _get_nc():
    if "nc" not in _NC_CACHE:
        _NC_CACHE["nc"] = build_nc()
    return _NC_CACHE["nc"]


def kernel(x, W, U, b):
    from concourse.bass_utils import run_bass_kernel_spmd

    nc = _get_nc()
    x = np.ascontiguousarray(np.asarray(x, np.float32))
    in_maps = [
        {
            "x": x[i * BL:(i + 1) * BL].reshape(ROWS, D),
            "W": np.asarray(W, np.float32),
            "U": np.asarray(U, np.float32),
            "b": np.asarray(b, np.float32).reshape(1, K),
        }
        for i in range(NCORES)
    ]
    res = run_bass_kernel_spmd(nc, in_maps, list(range(NCORES)),
                               trace=os.environ.get("CRF_TRACE", "") == "1")
    out = np.concatenate(
        [res.results[i]["out"].reshape(BL, T, K) for i in range(NCORES)], axis=0)
    return out


if __name__ == "__main__":
    xs = np.random.randn(B, T, D).astype(np.float32)
    Ws = (np.random.randn(D, K) / np.sqrt(D)).astype(np.float32)
    Us = (np.random.randn(K, K) * 0.1).astype(np.float32)
    bs = np.zeros(K, np.float32)
    o = kernel(xs, Ws, Us, bs)
    print(o.shape, o.dtype, o[0, 0, :4])
